# revision 15
# baseline (speedup 1.0000x reference)
"""Trainium2 Bass kernel for nn_MultiHeadAttention (dense transformer MHA).

Strategy (8-way tensor parallel over heads, v3 pipeline):
  - Each of the 8 cores owns 2 heads (128 of the 1024 q/k/v features).
  - Host pre-transposes activations to [D, T] bf16; weights head-sliced
    per core (Wo full). Inputs stream as per-d-tile slabs (4KB partition
    lines) over two DMA rings: sync carries wk/k/ropes/v-half then all
    output-side traffic (emitted as chores, so queue order interleaves);
    gpsimd (SWDGE) carries wq/q/wv/v-half/wo plus the collectives. The
    scalar ring hosts NO DMA so the ACT queue is pure exp work.
  - RoPE is elementwise here (reference uses neg_half=[y1,-y2]):
    rope(y)[t,f] = y[t,f] * C[f,t], C precomputed on host.
  - Attention in transposed layout S^T[s,t]. Softmax is unnormalized exp;
    the denominator rides as a ones-row appended to V' (row 64 of U).
    exp runs on the Scalar engine for 10/16 s-tiles and on the Vector
    engine for 6/16 via a Schraudolph bf16 bit-trick (affine + f32->int16
    round, bitcast as bf16), splitting the softmax load across engines.
  - Normalization: 1/denom (DVE fast reciprocal) broadcast across 64
    partitions via a ones-weight matmul; aT = U(PSUM) * rbc in one DVE
    pass. PSUM->SBUF staging copies ride the Scalar engine.
  - Per 512-chunk AllToAll re-partitions head-shards to row-shards; each
    core projects its 64 rows/chunk through full Wo with COLUMN-PAIRED
    matmuls (two concurrent M=64 matmuls at col strips 0/64).
  - PE warm-up matmuls chained to each k-slab DMA keep the HAM clock
    gate at 8/8 through the DMA prologue; chores are pinned to explicit
    s-indices so the PE queue never head-of-line blocks on DMA or
    collective latency.
"""
import numpy as np
import ml_dtypes

import concourse.bass as bass
import concourse.mybir as mybir
import concourse.tile as tile
from concourse import bacc
from concourse.bass_utils import run_bass_kernel_spmd

# problem constants (hardcoded per contract)
T = 2048
D = 1024
H = 16
DH = 64
ROPE_BASE = 10000

N_CORES = 8
HPC = H // N_CORES          # heads per core = 2
FPC = HPC * DH              # features per core = 128
TC = 512                    # attention t-chunk
NTC = T // TC               # 4
NS = T // 128               # 16 s-tiles
ND = D // 128               # 8 d-tiles
VW = 2 * (DH + 1)           # 130: v_ext block width per s-tile
ROWS = TC // N_CORES        # 64 output rows per core per A2A chunk
LAG = 4                     # u_mms trails exp by LAG s-tiles

bf16 = mybir.dt.bfloat16
f32 = mybir.dt.float32
i16 = mybir.dt.int16
EXP = mybir.ActivationFunctionType.Exp

# Schraudolph bf16 exp: bitpattern u = A*x + B with x = raw logit
LOG2E = 1.4426950408889634
SCH_A = 128.0 * LOG2E * 0.125          # fold the 1/sqrt(DH)=1/8 scale
SCH_B = 128.0 * 127.0 - 4.8            # shift minimizing rms rel err

_cache = {}


def _build(use_bias, n_dve):
    """n_dve: s-tiles per chunk whose exp runs on the Vector engine via
    the Schraudolph bit-trick (0 = all-exact on ScalarE)."""
    dve_set = set()
    if n_dve > 0:
        step = NS / n_dve
        dve_set = {int(step * i + step / 2) for i in range(n_dve)}

    nc = bacc.Bacc("TRN2", target_bir_lowering=False, debug=False,
                   num_devices=N_CORES)

    # ---- I/O -----------------------------------------------------------
    qT = nc.dram_tensor("qT", [D, T], bf16, kind="ExternalInput").ap()
    q0x = nc.dram_tensor("q0x", [D, TC], bf16, kind="ExternalInput").ap()
    kT = nc.dram_tensor("kT", [D, T], bf16, kind="ExternalInput").ap()
    vT = nc.dram_tensor("vT", [D, T], bf16, kind="ExternalInput").ap()
    wq = nc.dram_tensor("wq", [D, FPC], bf16, kind="ExternalInput").ap()
    wk = nc.dram_tensor("wk", [D, FPC], bf16, kind="ExternalInput").ap()
    wv = nc.dram_tensor("wv", [D, FPC], bf16, kind="ExternalInput").ap()
    wo = nc.dram_tensor("wo", [D, D], bf16, kind="ExternalInput").ap()
    bq = nc.dram_tensor("bq", [1, FPC], bf16, kind="ExternalInput").ap()
    bk = nc.dram_tensor("bk", [1, FPC], bf16, kind="ExternalInput").ap()
    bv = nc.dram_tensor("bv", [1, FPC], bf16, kind="ExternalInput").ap()
    bo = nc.dram_tensor("bo", [1, D], bf16, kind="ExternalInput").ap()
    ropeC = nc.dram_tensor("ropeC", [FPC, T], f32, kind="ExternalInput").ap()
    outs = [nc.dram_tensor(f"out{q}", [ROWS, D], f32,
                           kind="ExternalOutput").ap() for q in range(NTC)]

    with tile.TileContext(nc) as tc:
        with (
            tc.tile_pool(name="win", bufs=1) as win,        # weights/consts
            tc.tile_pool(name="xin", bufs=1) as xin,        # input slabs
            tc.tile_pool(name="qk", bufs=NTC) as qkpool,    # q^T / k^T
            tc.tile_pool(name="vx", bufs=NS) as vxpool,     # v_ext
            tc.tile_pool(name="ex", bufs=15) as expool,     # exp(S^T)
            tc.tile_pool(name="at", bufs=2) as atpool,      # attn^T halves
            tc.tile_pool(name="nrm", bufs=2) as nrmpool,    # rinv / rbc
            tc.tile_pool(name="opr", bufs=2) as oprpool,    # out-proj tiles
            tc.tile_pool(name="pp", bufs=2, space="PSUM") as pproj,
            tc.tile_pool(name="ps", bufs=2, space="PSUM") as pS,
            tc.tile_pool(name="pu", bufs=2, space="PSUM") as pU,
            tc.tile_pool(name="dram", bufs=1, space="DRAM") as dram,
        ):
            # ---- SBUF tiles -------------------------------------------
            wq_sb = win.tile([128, ND * FPC], bf16, tag="wq")
            wk_sb = win.tile([128, ND * FPC], bf16, tag="wk")
            wv_sb = win.tile([128, ND * FPC], bf16, tag="wv")
            bq_sb = win.tile([1, FPC], bf16, tag="bq")
            bk_sb = win.tile([1, FPC], bf16, tag="bk")
            bv_sb = win.tile([1, FPC], bf16, tag="bv")
            bo_sb = win.tile([1, D], bf16, tag="bo")
            wo_sb = win.tile([128, ND * D], bf16, tag="wo")
            ropes = [win.tile([FPC, TC], f32, tag="rope", bufs=NTC,
                              name=f"rope{i}") for i in range(NTC)]
            ones_sb = win.tile([1, TC], bf16, tag="ones")
            onesf_sb = win.tile([1, DH], f32, tag="onesf")
            scr_sb = win.tile([1, TC], bf16, tag="scr")
            qin = xin.tile([128, ND * T], bf16, tag="qin")
            kin = xin.tile([128, ND * T], bf16, tag="kin")
            vin = xin.tile([128, ND * T], bf16, tag="vin")
            qin0 = xin.tile([128, ND * TC], bf16, tag="qin0")
            qts = [qkpool.tile([128, TC], bf16, tag="qt", name=f"qt{i}")
                   for i in range(NTC)]
            kts = [qkpool.tile([128, TC], bf16, tag="kt", name=f"kt{i}")
                   for i in range(NTC)]
            vs = [vxpool.tile([128, VW], bf16, tag="vext", name=f"vext{s}")
                  for s in range(NS)]

            nc.gpsimd.memset(ones_sb[:], 1.0)
            nc.gpsimd.memset(onesf_sb[:], 1.0)
            for s in range(NS):
                nc.gpsimd.memset(vs[s][:, DH::DH + 1], 1.0)  # ones columns

            # preload the exp table set while DMAs stream
            nc.scalar.activation(scr_sb[0:1, 0:DH], ones_sb[0:1, 0:DH],
                                 EXP, scale=0.125)

            def _wdma(eng, w_sb, w):
                eng.dma_start(
                    out=w_sb[:].rearrange("p (d m) -> p d m", d=ND),
                    in_=w.rearrange("(d p) m -> p d m", p=128))

            def _xdma(eng, x_sb, x, d):
                eng.dma_start(
                    out=x_sb[:, T * d:T * (d + 1)],
                    in_=x[128 * d:128 * (d + 1), :])

            # q0x: duplicated chunk-0 columns of q so qt0 projects early
            def _q0xdma(eng, dlo, dhi):
                eng.dma_start(
                    out=qin0[:, TC * dlo:TC * dhi].rearrange(
                        "p (d t) -> p d t", d=dhi - dlo),
                    in_=q0x[128 * dlo:128 * dhi].rearrange(
                        "(d p) t -> p d t", p=128))

            # scalar ring (HWDGE; triggers enqueued before all exps):
            # q0x half, wk+wq, k half, v middle, q-slab tail
            _q0xdma(nc.scalar, 0, 4)
            _wdma(nc.scalar, wk_sb, wk)
            _wdma(nc.scalar, wq_sb, wq)
            if use_bias:
                nc.scalar.dma_start(out=bk_sb[:], in_=bk)
                nc.scalar.dma_start(out=bq_sb[:], in_=bq)
                nc.scalar.dma_start(out=bv_sb[:], in_=bv)
                nc.scalar.dma_start(out=bo_sb[:], in_=bo)
            for d in range(4):
                _xdma(nc.scalar, kin, kT, d)
            for d in range(3, 6):
                _xdma(nc.scalar, vin, vT, d)
            for d in range(4):
                _xdma(nc.scalar, qin, qT, d)

            # sync ring (HWDGE): q0x half, rope0, k half, v tail, q tail,
            # remaining ropes; all output-side traffic chores in later
            _q0xdma(nc.sync, 4, 8)
            nc.sync.dma_start(out=ropes[0][:], in_=ropeC[:, 0:TC])
            for d in range(4, ND):
                _xdma(nc.sync, kin, kT, d)
            for d in range(6, ND):
                _xdma(nc.sync, vin, vT, d)
            nc.sync.dma_start(out=ropes[1][:], in_=ropeC[:, TC:2 * TC])
            for d in range(4, ND):
                _xdma(nc.sync, qin, qT, d)
            for i in range(2, NTC):
                nc.sync.dma_start(out=ropes[i][:],
                                  in_=ropeC[:, TC * i:TC * (i + 1)])

            # gpsimd ring (SWDGE): wv + v head + wo; collectives later
            _wdma(nc.gpsimd, wv_sb, wv)
            for d in range(3):
                _xdma(nc.gpsimd, vin, vT, d)
            _wdma(nc.gpsimd, wo_sb, wo)

            # ---- PE warm-up: initial burst + keep-warm matmuls chained
            # to each k-slab arrival (~2.7us apart) hold HAM at 8/8.
            wup = pproj.tile([DH, TC], f32, tag="pp", name="wup")
            for i in range(8):
                nc.tensor.matmul(wup[:], ones_sb[:, 0:DH], ones_sb[:],
                                 start=(i == 0), stop=False)
            for d in range(ND):
                nc.tensor.matmul(wup[:], kin[0:1, T * d:T * d + DH],
                                 ones_sb[:], start=False, stop=(d == ND - 1))
            nc.vector.tensor_copy(scr_sb[:], wup[0:1, :])

            # ---- projections ------------------------------------------
            def proj_chunk(which, c):
                x_sb, w_sb, b_sb, x_in = {
                    "q": (qts[c], wq_sb, bq_sb, qin),
                    "k": (kts[c], wk_sb, bk_sb, kin),
                }[which]
                q0 = which == "q" and c == 0   # early dup, chunk-major tile
                ps = pproj.tile([128, TC], f32, tag="pp",
                                name=f"pj_{which}{c}")
                for d in range(ND):
                    rhs = (qin0[:, TC * d:TC * (d + 1)] if q0 else
                           x_in[:, T * d + TC * c:T * d + TC * (c + 1)])
                    nc.tensor.matmul(
                        ps[:], w_sb[:, FPC * d:FPC * (d + 1)], rhs,
                        start=(d == 0),
                        stop=(not use_bias and d == ND - 1))
                if use_bias:
                    nc.tensor.matmul(ps[:], b_sb[:], ones_sb[:],
                                     start=False, stop=True)
                nc.vector.tensor_mul(x_sb[:], ps[:], ropes[c][:])

            def vproj(s):
                ps = pproj.tile([128, FPC], f32, tag="pp", name=f"vps{s}")
                for d in range(ND):
                    nc.tensor.matmul(
                        ps[:], vin[:, T * d + 128 * s:T * d + 128 * (s + 1)],
                        wv_sb[:, FPC * d:FPC * (d + 1)],
                        start=(d == 0),
                        stop=(not use_bias and d == ND - 1))
                if use_bias:
                    nc.tensor.matmul(ps[:], ones_sb[:, 0:128], bv_sb[:],
                                     start=False, stop=True)
                nc.vector.tensor_copy(
                    vs[s][:].rearrange("p (h w) -> p h w", h=2)[:, :, 0:DH],
                    ps.rearrange("p (h w) -> p h w", h=2))

            proj_chunk("k", 0)
            proj_chunk("q", 0)

            # ---- A2A buffers ------------------------------------------
            a2a_in = [dram.tile([8 * 128, ROWS], bf16, tag=f"a2ai{i}",
                                name=f"a2a_in{i}") for i in range(NTC)]
            a2a_out = [dram.tile([8 * 128, ROWS], bf16, tag=f"a2ao{i}",
                                 name=f"a2a_out{i}") for i in range(NTC)]

            # ---- normalize + ship pieces ------------------------------
            nstate = {}

            def pbA(q):  # 1/denominator (stage via ScalarE, recip on DVE)
                up = nstate[q]["up"]
                rinv = nrmpool.tile([1, 2 * TC], f32, tag="rinv",
                                    name=f"rinv{q}")
                for h in range(HPC):
                    nc.scalar.copy(rinv[:, TC * h:TC * (h + 1)],
                                   up[h][DH:DH + 1, :])
                nc.vector.reciprocal_approx_fast(rinv[:], rinv[:])
                nstate[q]["rinv"] = rinv

            def pbB(q):  # broadcast 1/denom across 64 partitions
                rinv = nstate[q]["rinv"]
                rbcs = []
                for h in range(HPC):
                    rbp = pproj.tile([DH, TC], f32, tag="pp",
                                     name=f"rbp{q}_{h}")
                    nc.tensor.matmul(rbp[:], onesf_sb[:],
                                     rinv[:, TC * h:TC * (h + 1)],
                                     start=True, stop=True)
                    rbc = nrmpool.tile([DH, TC], f32, tag="rbc", bufs=4,
                                       name=f"rbc{q}_{h}")
                    nc.scalar.copy(rbc[:], rbp[:])
                    rbcs.append(rbc)
                nstate[q]["rbc"] = rbcs

            def pbC(q):  # aT = U * (1/denom), straight from PSUM
                up, rbcs = nstate[q]["up"], nstate[q]["rbc"]
                aTs = []
                for h in range(HPC):
                    aT = atpool.tile([DH, TC], bf16, tag=f"aT{h}",
                                     name=f"aTq{q}_{h}")
                    nc.vector.tensor_mul(aT[:], up[h][0:DH, :], rbcs[h][:])
                    aTs.append(aT)
                nstate[q]["aT"] = aTs

            def ship(q):
                aTs = nstate[q]["aT"]
                for h in range(HPC):
                    nc.sync.dma_start(
                        out=a2a_in[q].rearrange(
                            "(j h p) t -> h p j t", j=N_CORES, h=HPC)[h],
                        in_=aTs[h][:].rearrange("p (j t) -> p j t", j=N_CORES))
                nc.gpsimd.collective_compute(
                    "AllToAll", mybir.AluOpType.bypass,
                    replica_groups=[list(range(N_CORES))],
                    ins=[a2a_in[q][:].opt()],
                    outs=[a2a_out[q][:].opt()],
                )
                del nstate[q]

            # ---- output projection (column-paired) --------------------
            ostate = {}

            def op1(q):
                ap = oprpool.tile([128, ND * ROWS], bf16, tag="aprj",
                                  name=f"aprj{q}")
                nc.sync.dma_start(
                    out=ap[:].rearrange("p (d t) -> p d t", d=ND),
                    in_=a2a_out[q].rearrange("(d p) t -> p d t", p=128))
                ostate[q] = {"ap": ap}

            def _op_mms(q, ds):
                ap = ostate[q]["ap"]
                if "po" not in ostate[q]:
                    ostate[q]["po"] = pproj.tile([128, TC], f32, tag="pp",
                                                 name=f"po{q}")
                po = ostate[q]["po"]
                last = ds[-1] == ND - 1 and not use_bias
                for d in ds:
                    lw = ap[:, ROWS * d:ROWS * (d + 1)]
                    nc.tensor.matmul(
                        po[0:DH, :], lw, wo_sb[:, D * d:D * d + TC],
                        start=(d == 0), stop=(last and d == ds[-1]))
                    nc.tensor.matmul(
                        po[DH:128, :], lw, wo_sb[:, D * d + TC:D * d + D],
                        start=(d == 0), stop=(last and d == ds[-1]),
                        tile_position=(0, DH))
                if ds[-1] == ND - 1 and use_bias:
                    nc.tensor.matmul(po[0:DH, :], ones_sb[:, 0:ROWS],
                                     bo_sb[:, 0:TC], start=False, stop=True)
                    nc.tensor.matmul(po[DH:128, :], ones_sb[:, 0:ROWS],
                                     bo_sb[:, TC:D], start=False, stop=True,
                                     tile_position=(0, DH))

            def op2a(q):
                _op_mms(q, [0, 1, 2, 3])

            def op2b(q):
                _op_mms(q, [4, 5, 6, 7])

            def op3(q):
                po = ostate[q]["po"]
                oev = oprpool.tile([128, TC], f32, tag="oev", name=f"oev{q}")
                nc.vector.tensor_copy(oev[:], po[:])
                ostate[q]["oev"] = oev

            def op4(q):
                oev = ostate[q]["oev"]
                nc.sync.dma_start(out=outs[q][:, 0:TC], in_=oev[0:DH, :])
                nc.sync.dma_start(out=outs[q][:, TC:D], in_=oev[DH:128, :])
                del ostate[q]

            # ---- attention chunks -------------------------------------
            for c in range(NTC):
                if c == 0:
                    sched = {1: [lambda: proj_chunk("k", 1)],
                             4: [lambda: proj_chunk("k", 2)],
                             8: [lambda: proj_chunk("k", 3)],
                             12: [lambda: proj_chunk("q", 1)]}
                else:
                    q = c - 1
                    sched = {1: [lambda q=q: pbA(q)],
                             2: [lambda q=q: pbB(q)],
                             3: [lambda q=q: pbC(q)],
                             4: [lambda q=q: ship(q)]}
                    if c == 1:
                        sched[8] = [lambda: proj_chunk("q", 2)]
                    if c == 2:
                        sched[6] = [lambda: proj_chunk("q", 3)]
                    if c >= 2:
                        q2 = c - 2
                        sched[9] = [lambda q=q2: op1(q)]
                        sched[11] = [lambda q=q2: op2a(q)]
                        sched[12] = [lambda q=q2: op2b(q)]
                        sched[13] = [lambda q=q2: op3(q)]
                        sched[14] = [lambda q=q2: op4(q)]

                up = [pU.tile([DH + 1, TC], f32, tag="pu",
                              name=f"up{c}_{h}") for h in range(HPC)]
                nstate[c] = {"up": up}
                exq = []

                def u_mms(s, ex):
                    for h in range(HPC):
                        o = (DH + 1) * h
                        nc.tensor.matmul(
                            up[h][:], vs[s][:, o:o + DH + 1],
                            ex[:, TC * h:TC * (h + 1)],
                            start=(s == 0), stop=(s == NS - 1))

                for s in range(NS):
                    for fn in sched.get(s, ()):
                        fn()
                    kt_t = kts[s // 4]
                    ss = slice(128 * (s % 4), 128 * (s % 4 + 1))
                    sp = pS.tile([128, 2 * TC], f32, tag="ps")
                    nc.tensor.matmul(sp[:, 0:TC], kt_t[0:DH, ss],
                                     qts[c][0:DH, :], start=True, stop=True)
                    nc.tensor.matmul(sp[:, TC:2 * TC], kt_t[DH:128, ss],
                                     qts[c][DH:128, :], start=True,
                                     stop=True, tile_position=(DH, 0))
                    ex = expool.tile([128, 2 * TC], bf16, tag="ex")
                    if s in dve_set:
                        nc.vector.tensor_scalar(
                            out=ex[:].bitcast(i16), in0=sp[:],
                            scalar1=SCH_A, scalar2=SCH_B,
                            op0=mybir.AluOpType.mult,
                            op1=mybir.AluOpType.add)
                    else:
                        nc.scalar.activation(ex[:], sp[:], EXP, scale=0.125)
                    exq.append(ex)
                    if s >= LAG:
                        sl = s - LAG
                        if c == 0:
                            vproj(sl)
                        u_mms(sl, exq[sl])
                for sl in range(NS - LAG, NS):
                    if c == 0:
                        vproj(sl)
                    u_mms(sl, exq[sl])

            # drain
            pbA(NTC - 1), pbB(NTC - 1), pbC(NTC - 1), ship(NTC - 1)
            op1(NTC - 2), op2a(NTC - 2), op2b(NTC - 2)
            op3(NTC - 2), op4(NTC - 2)
            op1(NTC - 1), op2a(NTC - 1), op2b(NTC - 1)
            op3(NTC - 1), op4(NTC - 1)

    nc.compile()
    return nc


def _host_inputs(query, key, value, Wq, bq, Wk, bk, Wv, bv, Wo, bo):
    """Shard + lay out the full inputs for the 8 cores."""
    b = ml_dtypes.bfloat16
    qT = np.ascontiguousarray(query.T).astype(b)
    kT = np.ascontiguousarray(key.T).astype(b)
    vT = np.ascontiguousarray(value.T).astype(b)
    wo = Wo.astype(b)

    theta = 1.0 / (ROPE_BASE ** (np.arange(0, D, 2, dtype=np.float32) / D))
    idx = np.outer(np.arange(T, dtype=np.float32), theta)
    c, s = np.cos(idx), np.sin(idx)
    C = np.concatenate([c + s, c - s], axis=1).astype(np.float32)  # [T, D]

    in_maps = []
    for cidx in range(N_CORES):
        fs = slice(FPC * cidx, FPC * (cidx + 1))
        in_maps.append({
            "qT": qT, "kT": kT, "vT": vT,
            "q0x": np.ascontiguousarray(qT[:, 0:TC]),
            "wq": Wq[:, fs].astype(b), "wk": Wk[:, fs].astype(b),
            "wv": Wv[:, fs].astype(b), "wo": wo,
            "bq": bq[None, fs].astype(b), "bk": bk[None, fs].astype(b),
            "bv": bv[None, fs].astype(b), "bo": bo[None, :].astype(b),
            "ropeC": np.ascontiguousarray(C[:, fs].T),
        })
    return in_maps


N_DVE = 6  # s-tiles per chunk on the Vector-engine Schraudolph exp path


def kernel(query, key, value, Wq, bq, Wk, bk, Wv, bv, Wo, bo, _trace=False):
    query, key, value = (np.asarray(x, np.float32) for x in (query, key, value))
    Wq, Wk, Wv, Wo = (np.asarray(x, np.float32) for x in (Wq, Wk, Wv, Wo))
    bq, bk, bv, bo = (np.asarray(x, np.float32) for x in (bq, bk, bv, bo))
    use_bias = any(np.any(b) for b in (bq, bk, bv, bo))
    ck = f"nc{int(use_bias)}_{N_DVE}"
    if ck not in _cache:
        _cache[ck] = _build(use_bias, N_DVE)
    nc = _cache[ck]
    in_maps = _host_inputs(query, key, value, Wq, bq, Wk, bk, Wv, bv, Wo, bo)
    res = run_bass_kernel_spmd(nc, in_maps, core_ids=list(range(N_CORES)),
                               trace=_trace)
    _cache["last_result"] = res
    out = np.empty((T, D), np.float32)
    for c in range(N_CORES):
        for q in range(NTC):
            r0 = TC * q + ROWS * c
            out[r0:r0 + ROWS, :] = res.results[c][f"out{q}"]
    return out


# revision 22
# speedup vs baseline: 1.0626x; 1.0626x over previous
"""Trainium2 Bass kernel for nn_MultiHeadAttention (dense transformer MHA).

Strategy (8-way tensor parallel over heads, v3 pipeline):
  - Each of the 8 cores owns 2 heads (128 of the 1024 q/k/v features).
  - Host pre-transposes activations to [D, T] bf16; weights head-sliced
    per core (Wo full). Inputs stream as per-d-tile slabs (4KB partition
    lines) over two DMA rings: sync carries wk/k/ropes/v-half then all
    output-side traffic (emitted as chores, so queue order interleaves);
    gpsimd (SWDGE) carries wq/q/wv/v-half/wo plus the collectives. The
    scalar ring hosts NO DMA so the ACT queue is pure exp work.
  - RoPE is elementwise here (reference uses neg_half=[y1,-y2]):
    rope(y)[t,f] = y[t,f] * C[f,t], C precomputed on host.
  - Attention in transposed layout S^T[s,t]. Softmax is unnormalized exp;
    the denominator rides as a ones-row appended to V' (row 64 of U).
    exp runs on the Scalar engine for 10/16 s-tiles and on the Vector
    engine for 6/16 via a Schraudolph bf16 bit-trick (affine + f32->int16
    round, bitcast as bf16), splitting the softmax load across engines.
  - Normalization: 1/denom (DVE fast reciprocal) broadcast across 64
    partitions via a ones-weight matmul; aT = U(PSUM) * rbc in one DVE
    pass. PSUM->SBUF staging copies ride the Scalar engine.
  - Per 512-chunk AllToAll re-partitions head-shards to row-shards; each
    core projects its 64 rows/chunk through full Wo with COLUMN-PAIRED
    matmuls (two concurrent M=64 matmuls at col strips 0/64).
  - PE warm-up matmuls chained to each k-slab DMA keep the HAM clock
    gate at 8/8 through the DMA prologue; chores are pinned to explicit
    s-indices so the PE queue never head-of-line blocks on DMA or
    collective latency.
"""
import numpy as np
import ml_dtypes

import concourse.bass as bass
import concourse.mybir as mybir
import concourse.tile as tile
from concourse import bacc
from concourse.bass_utils import run_bass_kernel_spmd

# problem constants (hardcoded per contract)
T = 2048
D = 1024
H = 16
DH = 64
ROPE_BASE = 10000

N_CORES = 8
HPC = H // N_CORES          # heads per core = 2
FPC = HPC * DH              # features per core = 128
TC = 512                    # attention t-chunk
NTC = T // TC               # 4
NS = T // 128               # 16 s-tiles
ND = D // 128               # 8 d-tiles
VW = 2 * (DH + 1)           # 130: v_ext block width per s-tile
ROWS = TC // N_CORES        # 64 output rows per core per A2A chunk
LAG = 4                     # u_mms trails exp by LAG s-tiles

bf16 = mybir.dt.bfloat16
f32 = mybir.dt.float32
i16 = mybir.dt.int16
EXP = mybir.ActivationFunctionType.Exp

# Schraudolph bf16 exp: bitpattern u = A*x + B with x = raw logit
LOG2E = 1.4426950408889634
SCH_A = 128.0 * LOG2E * 0.125          # fold the 1/sqrt(DH)=1/8 scale
SCH_B = 128.0 * 127.0 - 4.8            # shift minimizing rms rel err

_cache = {}


def _build(use_bias, n_dve):
    """n_dve: s-tiles per chunk whose exp runs on the Vector engine via
    the Schraudolph bit-trick (0 = all-exact on ScalarE)."""
    dve_set = set()
    if n_dve > 0:
        step = NS / n_dve
        dve_set = {int(step * i + step / 2) for i in range(n_dve)}

    nc = bacc.Bacc("TRN2", target_bir_lowering=False, debug=False,
                   num_devices=N_CORES)

    # ---- I/O -----------------------------------------------------------
    qx = nc.dram_tensor("qx", [NTC, D, TC], bf16, kind="ExternalInput").ap()
    kT = nc.dram_tensor("kT", [D, T], bf16, kind="ExternalInput").ap()
    vT = nc.dram_tensor("vT", [D, T], bf16, kind="ExternalInput").ap()
    wq = nc.dram_tensor("wq", [D, FPC], bf16, kind="ExternalInput").ap()
    wk = nc.dram_tensor("wk", [D, FPC], bf16, kind="ExternalInput").ap()
    wv = nc.dram_tensor("wv", [D, FPC], bf16, kind="ExternalInput").ap()
    wo = nc.dram_tensor("wo", [D, D], bf16, kind="ExternalInput").ap()
    bq = nc.dram_tensor("bq", [1, FPC], bf16, kind="ExternalInput").ap()
    bk = nc.dram_tensor("bk", [1, FPC], bf16, kind="ExternalInput").ap()
    bv = nc.dram_tensor("bv", [1, FPC], bf16, kind="ExternalInput").ap()
    bo = nc.dram_tensor("bo", [1, D], bf16, kind="ExternalInput").ap()
    ropeC = nc.dram_tensor("ropeC", [FPC, T], f32, kind="ExternalInput").ap()
    outs = [nc.dram_tensor(f"out{q}", [ROWS, D], f32,
                           kind="ExternalOutput").ap() for q in range(NTC)]

    with tile.TileContext(nc) as tc:
        with (
            tc.tile_pool(name="win", bufs=1) as win,        # weights/consts
            tc.tile_pool(name="xin", bufs=1) as xin,        # input slabs
            tc.tile_pool(name="qk", bufs=NTC) as qkpool,    # q^T / k^T
            tc.tile_pool(name="vx", bufs=NS) as vxpool,     # v_ext
            tc.tile_pool(name="ex", bufs=16) as expool,     # exp(S^T)
            tc.tile_pool(name="at", bufs=2) as atpool,      # attn^T halves
            tc.tile_pool(name="nrm", bufs=2) as nrmpool,    # rinv / rbc
            tc.tile_pool(name="opr", bufs=2) as oprpool,    # out-proj tiles
            tc.tile_pool(name="pp", bufs=2, space="PSUM") as pproj,
            tc.tile_pool(name="ps", bufs=2, space="PSUM") as pS,
            tc.tile_pool(name="pu", bufs=2, space="PSUM") as pU,
            tc.tile_pool(name="dram", bufs=1, space="DRAM") as dram,
        ):
            # ---- SBUF tiles -------------------------------------------
            wq_sb = win.tile([128, ND * FPC], bf16, tag="wq")
            wk_sb = win.tile([128, ND * FPC], bf16, tag="wk")
            wv_sb = win.tile([128, ND * FPC], bf16, tag="wv")
            bq_sb = win.tile([1, FPC], bf16, tag="bq")
            bk_sb = win.tile([1, FPC], bf16, tag="bk")
            bv_sb = win.tile([1, FPC], bf16, tag="bv")
            bo_sb = win.tile([1, D], bf16, tag="bo")
            wo_sb = win.tile([128, ND * D], bf16, tag="wo")
            ropes = [win.tile([FPC, TC], f32, tag="rope", bufs=NTC,
                              name=f"rope{i}") for i in range(NTC)]
            ones_sb = win.tile([1, TC], bf16, tag="ones")
            onesf_sb = win.tile([1, DH], f32, tag="onesf")
            scr_sb = win.tile([1, TC], bf16, tag="scr")
            kin = xin.tile([128, ND * T], bf16, tag="kin")
            vin = xin.tile([128, ND * T], bf16, tag="vin")
            qcs = [xin.tile([128, ND * TC], bf16, tag=f"qc{c}",
                            name=f"qcs{c}") for c in range(NTC)]
            qts = [qkpool.tile([128, TC], bf16, tag="qt", name=f"qt{i}")
                   for i in range(NTC)]
            kts = [qkpool.tile([128, TC], bf16, tag="kt", name=f"kt{i}")
                   for i in range(NTC)]
            vs = [vxpool.tile([128, VW], bf16, tag="vext", name=f"vext{s}")
                  for s in range(NS)]

            nc.gpsimd.memset(ones_sb[:], 1.0)
            nc.gpsimd.memset(onesf_sb[:], 1.0)
            for s in range(NS):
                nc.gpsimd.memset(vs[s][:, DH::DH + 1], 1.0)  # ones columns

            # preload the exp table set while DMAs stream
            nc.scalar.activation(scr_sb[0:1, 0:DH], ones_sb[0:1, 0:DH],
                                 EXP, scale=0.125)

            def _wdma(eng, w_sb, w):
                eng.dma_start(
                    out=w_sb[:].rearrange("p (d m) -> p d m", d=ND),
                    in_=w.rearrange("(d p) m -> p d m", p=128))

            def _xdma(eng, x_sb, x, d):
                eng.dma_start(
                    out=x_sb[:, T * d:T * (d + 1)],
                    in_=x[128 * d:128 * (d + 1), :])

            # q-chunk DMA: qx[c] -> qcs[c], half the d-tiles per call
            def _qxdma(eng, c, dlo, dhi):
                eng.dma_start(
                    out=qcs[c][:, TC * dlo:TC * dhi].rearrange(
                        "p (d t) -> p d t", d=dhi - dlo),
                    in_=qx[c][128 * dlo:128 * dhi].rearrange(
                        "(d p) t -> p d t", p=128))

            # scalar ring (HWDGE): small early burst only, then the ACT
            # queue belongs to exp work.
            _qxdma(nc.scalar, 0, 0, 4)
            _wdma(nc.scalar, wk_sb, wk)
            _wdma(nc.scalar, wq_sb, wq)
            if use_bias:
                nc.scalar.dma_start(out=bk_sb[:], in_=bk)
                nc.scalar.dma_start(out=bq_sb[:], in_=bq)
                nc.scalar.dma_start(out=bv_sb[:], in_=bv)
                nc.scalar.dma_start(out=bo_sb[:], in_=bo)
            _xdma(nc.scalar, kin, kT, 0)
            _xdma(nc.scalar, kin, kT, 1)
            _xdma(nc.scalar, vin, vT, 4)
            _xdma(nc.scalar, vin, vT, 5)

            # sync ring (HWDGE): k/v tails, q1, wo; output chores later
            _qxdma(nc.sync, 0, 4, 8)
            nc.sync.dma_start(out=ropes[0][:], in_=ropeC[:, 0:TC])
            for d in range(2, 6):
                _xdma(nc.sync, kin, kT, d)
            _xdma(nc.sync, vin, vT, 6)
            _xdma(nc.sync, vin, vT, 7)
            nc.sync.dma_start(out=ropes[1][:], in_=ropeC[:, TC:2 * TC])
            _qxdma(nc.sync, 1, 0, 8)
            for i in range(2, NTC):
                nc.sync.dma_start(out=ropes[i][:],
                                  in_=ropeC[:, TC * i:TC * (i + 1)])
            _wdma(nc.sync, wo_sb, wo)

            # gpsimd ring (SWDGE): k/v heads, q2/q3; collectives later
            _xdma(nc.gpsimd, kin, kT, 6)
            _xdma(nc.gpsimd, kin, kT, 7)
            _wdma(nc.gpsimd, wv_sb, wv)
            for d in range(4):
                _xdma(nc.gpsimd, vin, vT, d)
            _qxdma(nc.gpsimd, 2, 0, 8)
            _qxdma(nc.gpsimd, 3, 0, 8)

            # ---- PE warm-up: initial burst + keep-warm matmuls chained
            # to each k-slab arrival (~2.7us apart) hold HAM at 8/8.
            wup = pproj.tile([DH, TC], f32, tag="pp", name="wup")
            for i in range(8):
                nc.tensor.matmul(wup[:], ones_sb[:, 0:DH], ones_sb[:],
                                 start=(i == 0), stop=False)
            for d in range(ND):
                nc.tensor.matmul(wup[:], kin[0:1, T * d:T * d + DH],
                                 ones_sb[:], start=False, stop=(d == ND - 1))
            nc.vector.tensor_copy(scr_sb[:], wup[0:1, :])

            # ---- projections ------------------------------------------
            def proj_chunk(which, c):
                x_sb, w_sb, b_sb = {
                    "q": (qts[c], wq_sb, bq_sb),
                    "k": (kts[c], wk_sb, bk_sb),
                }[which]
                ps = pproj.tile([128, TC], f32, tag="pp",
                                name=f"pj_{which}{c}")
                for d in range(ND):
                    rhs = (qcs[c][:, TC * d:TC * (d + 1)]
                           if which == "q" else
                           kin[:, T * d + TC * c:T * d + TC * (c + 1)])
                    nc.tensor.matmul(
                        ps[:], w_sb[:, FPC * d:FPC * (d + 1)], rhs,
                        start=(d == 0),
                        stop=(not use_bias and d == ND - 1))
                if use_bias:
                    nc.tensor.matmul(ps[:], b_sb[:], ones_sb[:],
                                     start=False, stop=True)
                nc.vector.tensor_mul(x_sb[:], ps[:], ropes[c][:])

            def vproj(s):
                ps = pproj.tile([128, FPC], f32, tag="pp", name=f"vps{s}")
                for d in range(ND):
                    nc.tensor.matmul(
                        ps[:], vin[:, T * d + 128 * s:T * d + 128 * (s + 1)],
                        wv_sb[:, FPC * d:FPC * (d + 1)],
                        start=(d == 0),
                        stop=(not use_bias and d == ND - 1))
                if use_bias:
                    nc.tensor.matmul(ps[:], ones_sb[:, 0:128], bv_sb[:],
                                     start=False, stop=True)
                nc.vector.tensor_copy(
                    vs[s][:].rearrange("p (h w) -> p h w", h=2)[:, :, 0:DH],
                    ps.rearrange("p (h w) -> p h w", h=2))

            proj_chunk("k", 0)
            proj_chunk("q", 0)

            # ---- A2A buffers ------------------------------------------
            a2a_in = [dram.tile([8 * 128, ROWS], bf16, tag=f"a2ai{i}",
                                name=f"a2a_in{i}") for i in range(NTC)]
            a2a_out = [dram.tile([8 * 128, ROWS], bf16, tag=f"a2ao{i}",
                                 name=f"a2a_out{i}") for i in range(NTC)]

            # ---- normalize + ship pieces ------------------------------
            nstate = {}

            def pbA(q):  # 1/denominator (stage via ScalarE, recip on DVE)
                up = nstate[q]["up"]
                rinv = nrmpool.tile([1, 2 * TC], f32, tag="rinv",
                                    name=f"rinv{q}")
                for h in range(HPC):
                    nc.scalar.copy(rinv[:, TC * h:TC * (h + 1)],
                                   up[h][DH:DH + 1, :])
                nc.vector.reciprocal_approx_fast(rinv[:], rinv[:])
                nstate[q]["rinv"] = rinv

            def pbB(q):  # broadcast 1/denom across 64 partitions
                rinv = nstate[q]["rinv"]
                rbcs = []
                for h in range(HPC):
                    rbp = pproj.tile([DH, TC], f32, tag="pp",
                                     name=f"rbp{q}_{h}")
                    nc.tensor.matmul(rbp[:], onesf_sb[:],
                                     rinv[:, TC * h:TC * (h + 1)],
                                     start=True, stop=True)
                    rbc = nrmpool.tile([DH, TC], f32, tag="rbc", bufs=4,
                                       name=f"rbc{q}_{h}")
                    nc.scalar.copy(rbc[:], rbp[:])
                    rbcs.append(rbc)
                nstate[q]["rbc"] = rbcs

            def pbC(q):  # aT = U * (1/denom), straight from PSUM
                up, rbcs = nstate[q]["up"], nstate[q]["rbc"]
                aTs = []
                for h in range(HPC):
                    aT = atpool.tile([DH, TC], bf16, tag=f"aT{h}",
                                     name=f"aTq{q}_{h}")
                    nc.vector.tensor_mul(aT[:], up[h][0:DH, :], rbcs[h][:])
                    aTs.append(aT)
                nstate[q]["aT"] = aTs

            def ship(q):
                aTs = nstate[q]["aT"]
                for h in range(HPC):
                    nc.sync.dma_start(
                        out=a2a_in[q].rearrange(
                            "(j h p) t -> h p j t", j=N_CORES, h=HPC)[h],
                        in_=aTs[h][:].rearrange("p (j t) -> p j t", j=N_CORES))
                nc.gpsimd.collective_compute(
                    "AllToAll", mybir.AluOpType.bypass,
                    replica_groups=[list(range(N_CORES))],
                    ins=[a2a_in[q][:].opt()],
                    outs=[a2a_out[q][:].opt()],
                )
                del nstate[q]

            # ---- output projection (column-paired) --------------------
            ostate = {}

            def op1(q):
                ap = oprpool.tile([128, ND * ROWS], bf16, tag="aprj",
                                  name=f"aprj{q}")
                nc.sync.dma_start(
                    out=ap[:].rearrange("p (d t) -> p d t", d=ND),
                    in_=a2a_out[q].rearrange("(d p) t -> p d t", p=128))
                ostate[q] = {"ap": ap}

            def _op_mms(q, ds):
                ap = ostate[q]["ap"]
                if "po" not in ostate[q]:
                    ostate[q]["po"] = pproj.tile([128, TC], f32, tag="pp",
                                                 name=f"po{q}")
                po = ostate[q]["po"]
                last = ds[-1] == ND - 1 and not use_bias
                for d in ds:
                    lw = ap[:, ROWS * d:ROWS * (d + 1)]
                    nc.tensor.matmul(
                        po[0:DH, :], lw, wo_sb[:, D * d:D * d + TC],
                        start=(d == 0), stop=(last and d == ds[-1]))
                    nc.tensor.matmul(
                        po[DH:128, :], lw, wo_sb[:, D * d + TC:D * d + D],
                        start=(d == 0), stop=(last and d == ds[-1]),
                        tile_position=(0, DH))
                if ds[-1] == ND - 1 and use_bias:
                    nc.tensor.matmul(po[0:DH, :], ones_sb[:, 0:ROWS],
                                     bo_sb[:, 0:TC], start=False, stop=True)
                    nc.tensor.matmul(po[DH:128, :], ones_sb[:, 0:ROWS],
                                     bo_sb[:, TC:D], start=False, stop=True,
                                     tile_position=(0, DH))

            def op2a(q):
                _op_mms(q, [0, 1, 2, 3])

            def op2b(q):
                _op_mms(q, [4, 5, 6, 7])

            def op3(q):
                po = ostate[q]["po"]
                oev = oprpool.tile([128, TC], f32, tag="oev", name=f"oev{q}")
                nc.vector.tensor_copy(oev[:], po[:])
                ostate[q]["oev"] = oev

            def op4(q):
                oev = ostate[q]["oev"]
                nc.sync.dma_start(out=outs[q][:, 0:TC], in_=oev[0:DH, :])
                nc.sync.dma_start(out=outs[q][:, TC:D], in_=oev[DH:128, :])
                del ostate[q]

            # ---- attention chunks -------------------------------------
            for c in range(NTC):
                if c == 0:
                    sched = {1: [lambda: proj_chunk("k", 1)],
                             4: [lambda: proj_chunk("k", 2)],
                             8: [lambda: proj_chunk("k", 3)],
                             12: [lambda: proj_chunk("q", 1)]}
                else:
                    q = c - 1
                    sched = {1: [lambda q=q: pbA(q)],
                             2: [lambda q=q: pbB(q)],
                             3: [lambda q=q: pbC(q)],
                             4: [lambda q=q: ship(q)]}
                    if c == 1:
                        sched[8] = [lambda: proj_chunk("q", 2)]
                    if c == 2:
                        sched[6] = [lambda: proj_chunk("q", 3)]
                    if c >= 2:
                        q2 = c - 2
                        sched[9] = [lambda q=q2: op1(q)]
                        sched[11] = [lambda q=q2: op2a(q)]
                        sched[12] = [lambda q=q2: op2b(q)]
                        sched[13] = [lambda q=q2: op3(q)]
                        sched[14] = [lambda q=q2: op4(q)]

                up = [pU.tile([DH + 1, TC], f32, tag="pu",
                              name=f"up{c}_{h}") for h in range(HPC)]
                nstate[c] = {"up": up}
                exq = []

                def u_mms(s, ex):
                    for h in range(HPC):
                        o = (DH + 1) * h
                        nc.tensor.matmul(
                            up[h][:], vs[s][:, o:o + DH + 1],
                            ex[:, TC * h:TC * (h + 1)],
                            start=(s == 0), stop=(s == NS - 1))

                for s in range(NS):
                    for fn in sched.get(s, ()):
                        fn()
                    kt_t = kts[s // 4]
                    ss = slice(128 * (s % 4), 128 * (s % 4 + 1))
                    sp = pS.tile([128, 2 * TC], f32, tag="ps")
                    nc.tensor.matmul(sp[:, 0:TC], kt_t[0:DH, ss],
                                     qts[c][0:DH, :], start=True, stop=True)
                    nc.tensor.matmul(sp[:, TC:2 * TC], kt_t[DH:128, ss],
                                     qts[c][DH:128, :], start=True,
                                     stop=True, tile_position=(DH, 0))
                    ex = expool.tile([128, 2 * TC], bf16, tag="ex")
                    if s in dve_set:
                        nc.vector.tensor_scalar(
                            out=ex[:].bitcast(i16), in0=sp[:],
                            scalar1=SCH_A, scalar2=SCH_B,
                            op0=mybir.AluOpType.mult,
                            op1=mybir.AluOpType.add)
                    else:
                        nc.scalar.activation(ex[:], sp[:], EXP, scale=0.125)
                    exq.append(ex)
                    if s >= LAG:
                        sl = s - LAG
                        if c == 0:
                            vproj(sl)
                        u_mms(sl, exq[sl])
                for sl in range(NS - LAG, NS):
                    if c == 0:
                        vproj(sl)
                    u_mms(sl, exq[sl])

            # drain
            pbA(NTC - 1), pbB(NTC - 1), pbC(NTC - 1), ship(NTC - 1)
            op1(NTC - 2), op2a(NTC - 2), op2b(NTC - 2)
            op3(NTC - 2), op4(NTC - 2)
            op1(NTC - 1), op2a(NTC - 1), op2b(NTC - 1)
            op3(NTC - 1), op4(NTC - 1)

    nc.compile()
    return nc


def _host_inputs(query, key, value, Wq, bq, Wk, bk, Wv, bv, Wo, bo):
    """Shard + lay out the full inputs for the 8 cores."""
    b = ml_dtypes.bfloat16
    qT = np.ascontiguousarray(query.T).astype(b)
    qx = np.ascontiguousarray(qT.reshape(D, NTC, TC).transpose(1, 0, 2))
    kT = np.ascontiguousarray(key.T).astype(b)
    vT = np.ascontiguousarray(value.T).astype(b)
    wo = Wo.astype(b)

    theta = 1.0 / (ROPE_BASE ** (np.arange(0, D, 2, dtype=np.float32) / D))
    idx = np.outer(np.arange(T, dtype=np.float32), theta)
    c, s = np.cos(idx), np.sin(idx)
    C = np.concatenate([c + s, c - s], axis=1).astype(np.float32)  # [T, D]

    in_maps = []
    for cidx in range(N_CORES):
        fs = slice(FPC * cidx, FPC * (cidx + 1))
        in_maps.append({
            "qx": qx, "kT": kT, "vT": vT,
            "wq": Wq[:, fs].astype(b), "wk": Wk[:, fs].astype(b),
            "wv": Wv[:, fs].astype(b), "wo": wo,
            "bq": bq[None, fs].astype(b), "bk": bk[None, fs].astype(b),
            "bv": bv[None, fs].astype(b), "bo": bo[None, :].astype(b),
            "ropeC": np.ascontiguousarray(C[:, fs].T),
        })
    return in_maps


N_DVE = 6  # s-tiles per chunk on the Vector-engine Schraudolph exp path


def kernel(query, key, value, Wq, bq, Wk, bk, Wv, bv, Wo, bo, _trace=False):
    query, key, value = (np.asarray(x, np.float32) for x in (query, key, value))
    Wq, Wk, Wv, Wo = (np.asarray(x, np.float32) for x in (Wq, Wk, Wv, Wo))
    bq, bk, bv, bo = (np.asarray(x, np.float32) for x in (bq, bk, bv, bo))
    use_bias = any(np.any(b) for b in (bq, bk, bv, bo))
    ck = f"nc{int(use_bias)}_{N_DVE}"
    if ck not in _cache:
        _cache[ck] = _build(use_bias, N_DVE)
    nc = _cache[ck]
    in_maps = _host_inputs(query, key, value, Wq, bq, Wk, bk, Wv, bv, Wo, bo)
    res = run_bass_kernel_spmd(nc, in_maps, core_ids=list(range(N_CORES)),
                               trace=_trace)
    _cache["last_result"] = res
    out = np.empty((T, D), np.float32)
    for c in range(N_CORES):
        for q in range(NTC):
            r0 = TC * q + ROWS * c
            out[r0:r0 + ROWS, :] = res.results[c][f"out{q}"]
    return out


# revision 33
# speedup vs baseline: 1.0719x; 1.0088x over previous
"""Trainium2 Bass kernel for nn_MultiHeadAttention (dense transformer MHA).

Strategy (8-way tensor parallel over heads, v3 pipeline):
  - Each of the 8 cores owns 2 heads (128 of the 1024 q/k/v features).
  - Host pre-transposes activations to [D, T] bf16; weights head-sliced
    per core (Wo full). Inputs stream as per-d-tile slabs (4KB partition
    lines) over two DMA rings: sync carries wk/k/ropes/v-half then all
    output-side traffic (emitted as chores, so queue order interleaves);
    gpsimd (SWDGE) carries wq/q/wv/v-half/wo plus the collectives. The
    scalar ring hosts NO DMA so the ACT queue is pure exp work.
  - RoPE is elementwise here (reference uses neg_half=[y1,-y2]):
    rope(y)[t,f] = y[t,f] * C[f,t], C precomputed on host.
  - Attention in transposed layout S^T[s,t]. Softmax is unnormalized exp;
    the denominator rides as a ones-row appended to V' (row 64 of U).
    exp runs on the Scalar engine for 10/16 s-tiles and on the Vector
    engine for 6/16 via a Schraudolph bf16 bit-trick (affine + f32->int16
    round, bitcast as bf16), splitting the softmax load across engines.
  - Normalization: 1/denom (DVE fast reciprocal) broadcast across 64
    partitions via a ones-weight matmul; aT = U(PSUM) * rbc in one DVE
    pass. PSUM->SBUF staging copies ride the Scalar engine.
  - Per 512-chunk AllToAll re-partitions head-shards to row-shards; each
    core projects its 64 rows/chunk through full Wo with COLUMN-PAIRED
    matmuls (two concurrent M=64 matmuls at col strips 0/64).
  - PE warm-up matmuls chained to each k-slab DMA keep the HAM clock
    gate at 8/8 through the DMA prologue; chores are pinned to explicit
    s-indices so the PE queue never head-of-line blocks on DMA or
    collective latency.
"""
import numpy as np
import ml_dtypes

import concourse.bass as bass
import concourse.mybir as mybir
import concourse.tile as tile
from concourse import bacc
from concourse.bass_utils import run_bass_kernel_spmd

# problem constants (hardcoded per contract)
T = 2048
D = 1024
H = 16
DH = 64
ROPE_BASE = 10000

N_CORES = 8
HPC = H // N_CORES          # heads per core = 2
FPC = HPC * DH              # features per core = 128
TC = 512                    # attention t-chunk
NTC = T // TC               # 4
NS = T // 128               # 16 s-tiles
ND = D // 128               # 8 d-tiles
VW = 2 * (DH + 1)           # 130: v_ext block width per s-tile
ROWS = TC // N_CORES        # 64 output rows per core per A2A chunk
LAG = 4                     # u_mms trails exp by LAG s-tiles

bf16 = mybir.dt.bfloat16
f32 = mybir.dt.float32
i16 = mybir.dt.int16
EXP = mybir.ActivationFunctionType.Exp

# Schraudolph bf16 exp: bitpattern u = A*x + B with x = raw logit
LOG2E = 1.4426950408889634
SCH_A = 128.0 * LOG2E * 0.125          # fold the 1/sqrt(DH)=1/8 scale
SCH_B = 128.0 * 127.0 - 4.8            # shift minimizing rms rel err

_cache = {}


def _build(use_bias, n_dve):
    """n_dve: s-tiles per chunk whose exp runs on the Vector engine via
    the Schraudolph bit-trick (0 = all-exact on ScalarE)."""
    dve_set = set()
    if n_dve > 0:
        step = NS / n_dve
        dve_set = {int(step * i + step / 2) for i in range(n_dve)}

    nc = bacc.Bacc("TRN2", target_bir_lowering=False, debug=False,
                   num_devices=N_CORES)

    # ---- I/O -----------------------------------------------------------
    qx = nc.dram_tensor("qx", [NTC, D, TC], bf16, kind="ExternalInput").ap()
    kT = nc.dram_tensor("kT", [D, T], bf16, kind="ExternalInput").ap()
    vx = nc.dram_tensor("vx", [NTC, D, TC], bf16, kind="ExternalInput").ap()
    wq = nc.dram_tensor("wq", [D, FPC], bf16, kind="ExternalInput").ap()
    wk = nc.dram_tensor("wk", [D, FPC], bf16, kind="ExternalInput").ap()
    wv = nc.dram_tensor("wv", [D, FPC], bf16, kind="ExternalInput").ap()
    wo = nc.dram_tensor("wo", [D, D], bf16, kind="ExternalInput").ap()
    bq = nc.dram_tensor("bq", [1, FPC], bf16, kind="ExternalInput").ap()
    bk = nc.dram_tensor("bk", [1, FPC], bf16, kind="ExternalInput").ap()
    bv = nc.dram_tensor("bv", [1, FPC], bf16, kind="ExternalInput").ap()
    bo = nc.dram_tensor("bo", [1, D], bf16, kind="ExternalInput").ap()
    ropeC = nc.dram_tensor("ropeC", [FPC, T], f32, kind="ExternalInput").ap()
    outs = [nc.dram_tensor(f"out{q}", [ROWS, D], f32,
                           kind="ExternalOutput").ap() for q in range(NTC)]

    with tile.TileContext(nc) as tc:
        with (
            tc.tile_pool(name="win", bufs=1) as win,        # weights/consts
            tc.tile_pool(name="xin", bufs=1) as xin,        # input slabs
            tc.tile_pool(name="qk", bufs=NTC) as qkpool,    # q^T / k^T
            tc.tile_pool(name="vx", bufs=NS) as vxpool,     # v_ext
            tc.tile_pool(name="ex", bufs=16) as expool,     # exp(S^T)
            tc.tile_pool(name="at", bufs=2) as atpool,      # attn^T halves
            tc.tile_pool(name="nrm", bufs=2) as nrmpool,    # rinv / rbc
            tc.tile_pool(name="opr", bufs=2) as oprpool,    # out-proj tiles
            tc.tile_pool(name="pp", bufs=2, space="PSUM") as pproj,
            tc.tile_pool(name="ps", bufs=2, space="PSUM") as pS,
            tc.tile_pool(name="pu", bufs=2, space="PSUM") as pU,
            tc.tile_pool(name="dram", bufs=1, space="DRAM") as dram,
        ):
            # ---- SBUF tiles -------------------------------------------
            wq_sb = win.tile([128, ND * FPC], bf16, tag="wq")
            wk_sb = win.tile([128, ND * FPC], bf16, tag="wk")
            wv_sb = win.tile([128, ND * FPC], bf16, tag="wv")
            bq_sb = win.tile([1, FPC], bf16, tag="bq")
            bk_sb = win.tile([1, FPC], bf16, tag="bk")
            bv_sb = win.tile([1, FPC], bf16, tag="bv")
            bo_sb = win.tile([1, D], bf16, tag="bo")
            wo_sb = win.tile([128, ND * D], bf16, tag="wo")
            ropes = [win.tile([FPC, TC], f32, tag="rope", bufs=NTC,
                              name=f"rope{i}") for i in range(NTC)]
            ones_sb = win.tile([1, TC], bf16, tag="ones")
            onesf_sb = win.tile([1, DH], f32, tag="onesf")
            scr_sb = win.tile([1, TC], bf16, tag="scr")
            kin = xin.tile([128, ND * T], bf16, tag="kin")
            qcs = [xin.tile([128, ND * TC], bf16, tag=f"qc{c}",
                            name=f"qcs{c}") for c in range(NTC)]
            vcs = [xin.tile([128, ND * TC], bf16, tag=f"vc{c}",
                            name=f"vcs{c}") for c in range(NTC)]
            qts = [qkpool.tile([128, TC], bf16, tag="qt", name=f"qt{i}")
                   for i in range(NTC)]
            kts = [qkpool.tile([128, TC], bf16, tag="kt", name=f"kt{i}")
                   for i in range(NTC)]
            vs = [vxpool.tile([128, VW], bf16, tag="vext", name=f"vext{s}")
                  for s in range(NS)]

            nc.gpsimd.memset(ones_sb[:], 1.0)
            nc.gpsimd.memset(onesf_sb[:], 1.0)
            for s in range(NS):
                nc.gpsimd.memset(vs[s][:, DH::DH + 1], 1.0)  # ones columns

            # preload the exp table set while DMAs stream
            nc.scalar.activation(scr_sb[0:1, 0:DH], ones_sb[0:1, 0:DH],
                                 EXP, scale=0.125)

            def _wdma(eng, w_sb, w):
                eng.dma_start(
                    out=w_sb[:].rearrange("p (d m) -> p d m", d=ND),
                    in_=w.rearrange("(d p) m -> p d m", p=128))

            def _xdma(eng, x_sb, x, d):
                eng.dma_start(
                    out=x_sb[:, T * d:T * (d + 1)],
                    in_=x[128 * d:128 * (d + 1), :])

            # chunk DMA: src[c] -> dst[c] ([NTC, D, TC] chunk tensors)
            def _cxdma(eng, dst, src, c):
                eng.dma_start(
                    out=dst[c][:].rearrange("p (d t) -> p d t", d=ND),
                    in_=src[c].rearrange("(d p) t -> p d t", p=128))

            # scalar ring (HWDGE): early burst only (k head + weights +
            # q0/q1), then the ACT queue belongs to exp work.
            _xdma(nc.scalar, kin, kT, 0)
            _xdma(nc.scalar, kin, kT, 1)
            _xdma(nc.scalar, kin, kT, 2)
            _wdma(nc.scalar, wk_sb, wk)
            if use_bias:
                nc.scalar.dma_start(out=bk_sb[:], in_=bk)
                nc.scalar.dma_start(out=bq_sb[:], in_=bq)
                nc.scalar.dma_start(out=bv_sb[:], in_=bv)
                nc.scalar.dma_start(out=bo_sb[:], in_=bo)
            _cxdma(nc.scalar, qcs, qx, 1)

            # sync ring (HWDGE): k tail, q0, v1/v3; wo + output-side
            # traffic rides later as chores (queue order = emission).
            _xdma(nc.sync, kin, kT, 3)
            _xdma(nc.sync, kin, kT, 4)
            _xdma(nc.sync, kin, kT, 5)
            nc.sync.dma_start(out=ropes[0][:], in_=ropeC[:, 0:TC])
            _wdma(nc.sync, wq_sb, wq)
            _cxdma(nc.sync, qcs, qx, 0)
            _cxdma(nc.sync, vcs, vx, 1)
            nc.sync.dma_start(out=ropes[1][:], in_=ropeC[:, TC:2 * TC])
            _cxdma(nc.sync, vcs, vx, 3)
            for i in range(2, NTC):
                nc.sync.dma_start(out=ropes[i][:],
                                  in_=ropeC[:, TC * i:TC * (i + 1)])

            def _wodma(part):
                nc.sync.dma_start(
                    out=wo_sb[:, D * 2 * part:D * 2 * (part + 1)].rearrange(
                        "p (d m) -> p d m", d=2),
                    in_=wo[256 * part:256 * (part + 1)].rearrange(
                        "(d p) m -> p d m", p=128))

            # gpsimd ring (SWDGE): k head, wv, v0/v2, q2/q3
            _xdma(nc.gpsimd, kin, kT, 6)
            _xdma(nc.gpsimd, kin, kT, 7)
            _wdma(nc.gpsimd, wv_sb, wv)
            _cxdma(nc.gpsimd, vcs, vx, 0)
            _cxdma(nc.gpsimd, vcs, vx, 2)
            _cxdma(nc.gpsimd, qcs, qx, 2)
            _cxdma(nc.gpsimd, qcs, qx, 3)

            # ---- PE warm-up: initial burst + keep-warm matmuls chained
            # to each k-slab arrival (~2.7us apart) hold HAM at 8/8.
            wup = pproj.tile([DH, TC], f32, tag="pp", name="wup")
            for i in range(8):
                nc.tensor.matmul(wup[:], ones_sb[:, 0:DH], ones_sb[:],
                                 start=(i == 0), stop=False)
            for d in range(ND):
                nc.tensor.matmul(wup[:], kin[0:1, T * d:T * d + DH],
                                 ones_sb[:], start=False, stop=(d == ND - 1))
            nc.vector.tensor_copy(scr_sb[:], wup[0:1, :])

            # ---- projections ------------------------------------------
            def proj_chunk(which, c):
                x_sb, w_sb, b_sb = {
                    "q": (qts[c], wq_sb, bq_sb),
                    "k": (kts[c], wk_sb, bk_sb),
                }[which]
                ps = pproj.tile([128, TC], f32, tag="pp",
                                name=f"pj_{which}{c}")
                for d in range(ND):
                    rhs = (qcs[c][:, TC * d:TC * (d + 1)]
                           if which == "q" else
                           kin[:, T * d + TC * c:T * d + TC * (c + 1)])
                    nc.tensor.matmul(
                        ps[:], w_sb[:, FPC * d:FPC * (d + 1)], rhs,
                        start=(d == 0),
                        stop=(not use_bias and d == ND - 1))
                if use_bias:
                    nc.tensor.matmul(ps[:], b_sb[:], ones_sb[:],
                                     start=False, stop=True)
                nc.vector.tensor_mul(x_sb[:], ps[:], ropes[c][:])

            def vproj(s):
                vc, j = vcs[s // 4], s % 4
                ps = pproj.tile([128, FPC], f32, tag="pp", name=f"vps{s}")
                for d in range(ND):
                    nc.tensor.matmul(
                        ps[:], vc[:, TC * d + 128 * j:TC * d + 128 * (j + 1)],
                        wv_sb[:, FPC * d:FPC * (d + 1)],
                        start=(d == 0),
                        stop=(not use_bias and d == ND - 1))
                if use_bias:
                    nc.tensor.matmul(ps[:], ones_sb[:, 0:128], bv_sb[:],
                                     start=False, stop=True)
                nc.vector.tensor_copy(
                    vs[s][:].rearrange("p (h w) -> p h w", h=2)[:, :, 0:DH],
                    ps.rearrange("p (h w) -> p h w", h=2))

            proj_chunk("k", 0)
            proj_chunk("q", 0)

            # ---- A2A buffers ------------------------------------------
            a2a_in = [dram.tile([8 * 128, ROWS], bf16, tag=f"a2ai{i}",
                                name=f"a2a_in{i}") for i in range(NTC)]
            a2a_out = [dram.tile([8 * 128, ROWS], bf16, tag=f"a2ao{i}",
                                 name=f"a2a_out{i}") for i in range(NTC)]

            # ---- normalize + ship pieces ------------------------------
            nstate = {}

            def pbA(q):  # 1/denominator (stage via ScalarE, recip on DVE)
                up = nstate[q]["up"]
                rinv = nrmpool.tile([1, 2 * TC], f32, tag="rinv",
                                    name=f"rinv{q}")
                for h in range(HPC):
                    nc.scalar.copy(rinv[:, TC * h:TC * (h + 1)],
                                   up[h][DH:DH + 1, :])
                nc.vector.reciprocal_approx_fast(rinv[:], rinv[:])
                nstate[q]["rinv"] = rinv

            def pbB(q):  # broadcast 1/denom across 64 partitions
                rinv = nstate[q]["rinv"]
                rbcs = []
                for h in range(HPC):
                    rbp = pproj.tile([DH, TC], f32, tag="pp",
                                     name=f"rbp{q}_{h}")
                    nc.tensor.matmul(rbp[:], onesf_sb[:],
                                     rinv[:, TC * h:TC * (h + 1)],
                                     start=True, stop=True)
                    rbc = nrmpool.tile([DH, TC], f32, tag="rbc", bufs=4,
                                       name=f"rbc{q}_{h}")
                    nc.scalar.copy(rbc[:], rbp[:])
                    rbcs.append(rbc)
                nstate[q]["rbc"] = rbcs

            def pbC(q):  # aT = U * (1/denom), straight from PSUM
                up, rbcs = nstate[q]["up"], nstate[q]["rbc"]
                aTs = []
                for h in range(HPC):
                    aT = atpool.tile([DH, TC], bf16, tag=f"aT{h}",
                                     name=f"aTq{q}_{h}")
                    nc.vector.tensor_mul(aT[:], up[h][0:DH, :], rbcs[h][:])
                    aTs.append(aT)
                nstate[q]["aT"] = aTs

            def ship(q):
                aTs = nstate[q]["aT"]
                for h in range(HPC):
                    nc.sync.dma_start(
                        out=a2a_in[q].rearrange(
                            "(j h p) t -> h p j t", j=N_CORES, h=HPC)[h],
                        in_=aTs[h][:].rearrange("p (j t) -> p j t", j=N_CORES))
                nc.gpsimd.collective_compute(
                    "AllToAll", mybir.AluOpType.bypass,
                    replica_groups=[list(range(N_CORES))],
                    ins=[a2a_in[q][:].opt()],
                    outs=[a2a_out[q][:].opt()],
                )
                del nstate[q]

            # ---- output projection (column-paired) --------------------
            ostate = {}

            def op1(q):
                ap = oprpool.tile([128, ND * ROWS], bf16, tag="aprj",
                                  name=f"aprj{q}")
                nc.sync.dma_start(
                    out=ap[:].rearrange("p (d t) -> p d t", d=ND),
                    in_=a2a_out[q].rearrange("(d p) t -> p d t", p=128))
                ostate[q] = {"ap": ap}

            def _op_mms(q, ds):
                ap = ostate[q]["ap"]
                if "po" not in ostate[q]:
                    ostate[q]["po"] = pproj.tile([128, TC], f32, tag="pp",
                                                 name=f"po{q}")
                po = ostate[q]["po"]
                last = ds[-1] == ND - 1 and not use_bias
                for d in ds:
                    lw = ap[:, ROWS * d:ROWS * (d + 1)]
                    nc.tensor.matmul(
                        po[0:DH, :], lw, wo_sb[:, D * d:D * d + TC],
                        start=(d == 0), stop=(last and d == ds[-1]))
                    nc.tensor.matmul(
                        po[DH:128, :], lw, wo_sb[:, D * d + TC:D * d + D],
                        start=(d == 0), stop=(last and d == ds[-1]),
                        tile_position=(0, DH))
                if ds[-1] == ND - 1 and use_bias:
                    nc.tensor.matmul(po[0:DH, :], ones_sb[:, 0:ROWS],
                                     bo_sb[:, 0:TC], start=False, stop=True)
                    nc.tensor.matmul(po[DH:128, :], ones_sb[:, 0:ROWS],
                                     bo_sb[:, TC:D], start=False, stop=True,
                                     tile_position=(0, DH))

            def op2a(q):
                _op_mms(q, [0, 1, 2, 3])

            def op2b(q):
                _op_mms(q, [4, 5, 6, 7])

            def op3(q):
                po = ostate[q]["po"]
                oev = oprpool.tile([128, TC], f32, tag="oev", name=f"oev{q}")
                nc.vector.tensor_copy(oev[:], po[:])
                ostate[q]["oev"] = oev

            def op4(q):
                oev = ostate[q]["oev"]
                nc.sync.dma_start(out=outs[q][:, 0:TC], in_=oev[0:DH, :])
                nc.sync.dma_start(out=outs[q][:, TC:D], in_=oev[DH:128, :])
                del ostate[q]

            # ---- attention chunks -------------------------------------
            for c in range(NTC):
                if c == 0:
                    sched = {1: [lambda: proj_chunk("k", 1)],
                             4: [lambda: proj_chunk("k", 2)],
                             8: [lambda: proj_chunk("k", 3)],
                             12: [lambda: proj_chunk("q", 1)]}
                else:
                    q = c - 1
                    sched = {1: [lambda q=q: pbA(q)],
                             2: [lambda q=q: pbB(q)],
                             3: [lambda q=q: pbC(q)],
                             4: [lambda q=q: ship(q)]}
                    if c == 1:
                        sched[6] = [lambda: _wodma(0)]
                        sched[9] = [lambda: _wodma(1)]
                        sched[12] = [lambda: proj_chunk("q", 2),
                                     lambda: _wodma(2)]
                    if c == 2:
                        sched[5] = [lambda: _wodma(3)]
                        sched[6] = [lambda: proj_chunk("q", 3)]
                        sched[9] = [lambda: op1(0)]
                    if c == 3:
                        sched[9] = [lambda: op1(1)]

                up = [pU.tile([DH + 1, TC], f32, tag="pu",
                              name=f"up{c}_{h}") for h in range(HPC)]
                nstate[c] = {"up": up}
                exq = []

                def u_mms(s, ex):
                    for h in range(HPC):
                        o = (DH + 1) * h
                        nc.tensor.matmul(
                            up[h][:], vs[s][:, o:o + DH + 1],
                            ex[:, TC * h:TC * (h + 1)],
                            start=(s == 0), stop=(s == NS - 1))

                for s in range(NS):
                    for fn in sched.get(s, ()):
                        fn()
                    kt_t = kts[s // 4]
                    ss = slice(128 * (s % 4), 128 * (s % 4 + 1))
                    sp = pS.tile([128, 2 * TC], f32, tag="ps")
                    nc.tensor.matmul(sp[:, 0:TC], kt_t[0:DH, ss],
                                     qts[c][0:DH, :], start=True, stop=True)
                    nc.tensor.matmul(sp[:, TC:2 * TC], kt_t[DH:128, ss],
                                     qts[c][DH:128, :], start=True,
                                     stop=True, tile_position=(DH, 0))
                    ex = expool.tile([128, 2 * TC], bf16, tag="ex")
                    if s in dve_set:
                        nc.vector.tensor_scalar(
                            out=ex[:].bitcast(i16), in0=sp[:],
                            scalar1=SCH_A, scalar2=SCH_B,
                            op0=mybir.AluOpType.mult,
                            op1=mybir.AluOpType.add)
                    else:
                        nc.scalar.activation(ex[:], sp[:], EXP, scale=0.125)
                    exq.append(ex)
                    if s >= LAG:
                        sl = s - LAG
                        if c == 0:
                            vproj(sl)
                        u_mms(sl, exq[sl])
                for sl in range(NS - LAG, NS):
                    if c == 0:
                        vproj(sl)
                    u_mms(sl, exq[sl])

            # drain: normalize+ship chunk 3, then ALL out-projections
            # (A2A-dependent PE work never blocks the attention window)
            pbA(NTC - 1), pbB(NTC - 1), pbC(NTC - 1), ship(NTC - 1)
            op1(2), op1(3)
            for q in range(NTC):
                op2a(q), op2b(q), op3(q), op4(q)

    nc.compile()
    return nc


def _host_inputs(query, key, value, Wq, bq, Wk, bk, Wv, bv, Wo, bo):
    """Shard + lay out the full inputs for the 8 cores."""
    b = ml_dtypes.bfloat16
    def chunked(x):
        xT = np.ascontiguousarray(x.T).astype(b)
        return np.ascontiguousarray(
            xT.reshape(D, NTC, TC).transpose(1, 0, 2))

    qx = chunked(query)
    vx = chunked(value)
    kT = np.ascontiguousarray(key.T).astype(b)
    wo = Wo.astype(b)

    theta = 1.0 / (ROPE_BASE ** (np.arange(0, D, 2, dtype=np.float32) / D))
    idx = np.outer(np.arange(T, dtype=np.float32), theta)
    c, s = np.cos(idx), np.sin(idx)
    C = np.concatenate([c + s, c - s], axis=1).astype(np.float32)  # [T, D]

    in_maps = []
    for cidx in range(N_CORES):
        fs = slice(FPC * cidx, FPC * (cidx + 1))
        in_maps.append({
            "qx": qx, "kT": kT, "vx": vx,
            "wq": Wq[:, fs].astype(b), "wk": Wk[:, fs].astype(b),
            "wv": Wv[:, fs].astype(b), "wo": wo,
            "bq": bq[None, fs].astype(b), "bk": bk[None, fs].astype(b),
            "bv": bv[None, fs].astype(b), "bo": bo[None, :].astype(b),
            "ropeC": np.ascontiguousarray(C[:, fs].T),
        })
    return in_maps


N_DVE = 6  # s-tiles per chunk on the Vector-engine Schraudolph exp path


def kernel(query, key, value, Wq, bq, Wk, bk, Wv, bv, Wo, bo, _trace=False):
    query, key, value = (np.asarray(x, np.float32) for x in (query, key, value))
    Wq, Wk, Wv, Wo = (np.asarray(x, np.float32) for x in (Wq, Wk, Wv, Wo))
    bq, bk, bv, bo = (np.asarray(x, np.float32) for x in (bq, bk, bv, bo))
    use_bias = any(np.any(b) for b in (bq, bk, bv, bo))
    ck = f"nc{int(use_bias)}_{N_DVE}"
    if ck not in _cache:
        _cache[ck] = _build(use_bias, N_DVE)
    nc = _cache[ck]
    in_maps = _host_inputs(query, key, value, Wq, bq, Wk, bk, Wv, bv, Wo, bo)
    res = run_bass_kernel_spmd(nc, in_maps, core_ids=list(range(N_CORES)),
                               trace=_trace)
    _cache["last_result"] = res
    out = np.empty((T, D), np.float32)
    for c in range(N_CORES):
        for q in range(NTC):
            r0 = TC * q + ROWS * c
            out[r0:r0 + ROWS, :] = res.results[c][f"out{q}"]
    return out
